# revision 1
# baseline (speedup 1.0000x reference)
"""GQA attention block (16 query heads / 4 KV groups, head_dim 128) on 8 TRN2 NeuronCores.

Sharding: data-parallel over batch (b=2) x tensor-parallel over the 4 KV groups.
Core c handles batch c//4, KV group c%4 (4 query heads). Each core computes its
group's Q/K/V projections, causal softmax attention, and a partial out-projection
(row-shard of Wo); the host sums the 4 partials per batch and adds the bias.

All matmuls run in bf16 (fp32 PSUM accumulation). Host pre-transposes x to x^T
(and packs Wk/Wv partition-major) so every matmul operand is already in the
[K, M]/[K, N] layout the PE wants; the only on-chip transposes are the per-block
128x128 context-tile transposes ahead of the out-projection. Softmax runs without
the running-max (score scale is bounded by the input distribution); the denominator
comes from a ones-column appended to V.

Schedule: inputs arrive in 9 packed DMAs, x column-block-first. Work streams per
512-row query block — projections for block nq, then attention for query block nq
(which by causality only needs K/V blocks <= nq), then that block's out-projection
and output DMA. Projection matmuls of block nq+1 fill the PE bubbles left by the
exp->ctx latency chain of block nq.
"""

import math

import ml_dtypes
import numpy as np

B = 2
T = 2048
D_IN = 2048
N_KV = 4          # KV groups (one per core within a batch)
GH = 4            # query heads per KV group
HD = 128          # head dim
GD = GH * HD      # 512: per-group q/ctx width
TT = T // 128     # 16 row tiles
CC = D_IN // 128  # 16 contraction chunks
NQ = T // 512     # 4 query chunks of 512
SCALE = 1.0 / math.sqrt(HD)

_COMPILED = None


def _build():
    import concourse.bacc as bacc
    import concourse.tile as tile
    from concourse import mybir
    from concourse.masks import make_identity

    bf16 = mybir.dt.bfloat16
    f32 = mybir.dt.float32

    nc = bacc.Bacc("TRN2", target_bir_lowering=False, debug=False)

    # xT: x^T per batch; wk/wv packed partition-major on host: [128, c*HD]
    xT_d = nc.dram_tensor("xT", [D_IN, T], bf16, kind="ExternalInput")
    wq_d = nc.dram_tensor("wq", [D_IN, GD], bf16, kind="ExternalInput")
    wk_d = nc.dram_tensor("wk", [128, CC * HD], bf16, kind="ExternalInput")
    wv_d = nc.dram_tensor("wv", [128, CC * HD], bf16, kind="ExternalInput")
    wo_d = nc.dram_tensor("wo", [GD, D_IN], bf16, kind="ExternalInput")
    mask_d = nc.dram_tensor("mask", [128, 4 * 512], bf16, kind="ExternalInput")
    out_d = nc.dram_tensor("out", [T, D_IN], bf16, kind="ExternalOutput")

    with tile.TileContext(nc) as tc:
        with (
            tc.tile_pool(name="persist", bufs=1) as persist,
            tc.tile_pool(name="ptp", bufs=32) as ptp,
            tc.tile_pool(name="smalls", bufs=8) as smalls,
            tc.tile_pool(name="outsb", bufs=3) as outsb,
            tc.tile_pool(name="psum", bufs=2, space="PSUM") as psum,
        ):
            # ---- packed input DMAs, x column-block-first ----
            wk_all = persist.tile([128, CC, HD], bf16, name="wk_all", tag="wk_all")
            nc.sync.dma_start(
                out=wk_all, in_=wk_d.ap().rearrange("p (c n) -> p c n", c=CC)
            )
            xb = [
                persist.tile([128, CC, 512], bf16, name=f"xb{nq}", tag=f"xb{nq}")
                for nq in range(NQ)
            ]
            # split the first block's load so kT/v matmuls can start after
            # the first half arrives (subtile deps unblock c-chunks 0..7)
            nc.sync.dma_start(
                out=xb[0][:, 0:8, :],
                in_=xT_d[0:1024, 0:512].rearrange("(c p) n -> p c n", c=8),
            )
            nc.sync.dma_start(
                out=xb[0][:, 8:16, :],
                in_=xT_d[1024:2048, 0:512].rearrange("(c p) n -> p c n", c=8),
            )
            wv_all = persist.tile([128, CC, HD], bf16, name="wv_all", tag="wv_all")
            nc.sync.dma_start(
                out=wv_all, in_=wv_d.ap().rearrange("p (c n) -> p c n", c=CC)
            )
            wq_all = persist.tile([128, CC, GD], bf16, name="wq_all", tag="wq_all")
            # halves: heads 0-1 can project as soon as the first half lands
            nc.sync.dma_start(
                out=wq_all[:, :, 0:256],
                in_=wq_d[:, 0:256].rearrange("(c p) n -> p c n", c=CC),
            )
            nc.sync.dma_start(
                out=wq_all[:, :, 256:512],
                in_=wq_d[:, 256:512].rearrange("(c p) n -> p c n", c=CC),
            )
            mask_sb = persist.tile([128, 4 * 512], bf16, name="mask_sb", tag="mask_sb")
            nc.sync.dma_start(out=mask_sb, in_=mask_d[:, :])
            nc.sync.dma_start(
                out=xb[1], in_=xT_d[:, 512:1024].rearrange("(c p) n -> p c n", c=CC)
            )
            wo_all = persist.tile([128, GH, D_IN], bf16, name="wo_all", tag="wo_all")
            nc.sync.dma_start(
                out=wo_all, in_=wo_d.ap().rearrange("(h p) n -> p h n", h=GH)
            )
            for nq in range(2, NQ):
                nc.sync.dma_start(
                    out=xb[nq],
                    in_=xT_d[:, nq * 512:(nq + 1) * 512].rearrange(
                        "(c p) n -> p c n", c=CC
                    ),
                )
            identity = persist.tile([128, 128], bf16, name="identity", tag="identity")
            make_identity(nc, identity)

            kT_blk = [
                persist.tile([128, 512], bf16, name=f"kT{nq}", tag=f"kT{nq}")
                for nq in range(NQ)
            ]
            qT_blk = [
                [
                    persist.tile([128, 512], bf16, name=f"qT{h}_{nq}", tag=f"qT{h}_{nq}")
                    for nq in range(NQ)
                ]
                for h in range(GH)
            ]
            vext = [
                persist.tile([128, 132], bf16, name=f"vx{t}", tag=f"vx{t}")
                for t in range(TT)
            ]
            for t in range(TT):
                nc.vector.memset(vext[t][:, 128:129], 1.0)
            ctxT_blk = [
                [
                    persist.tile([128, 512], bf16, name=f"cT{h}_{nq}", tag=f"cT{h}_{nq}")
                    for nq in range(NQ)
                ]
                for h in range(GH)
            ]

            def emit_proj(nq):
                ps = psum.tile([128, 512], f32, name="pskt", tag="psP", bufs=2)
                for c in range(CC):
                    nc.tensor.matmul(
                        ps, wk_all[:, c, :], xb[nq][:, c, :],
                        start=(c == 0), stop=(c == CC - 1),
                    )
                nc.scalar.copy(out=kT_blk[nq], in_=ps)
                for ts in range(4):
                    t = nq * 4 + ts
                    pv = psum.tile([128, 512], f32, name="psv", tag="psP", bufs=2)
                    for c in range(CC):
                        nc.tensor.matmul(
                            pv[:, 0:128],
                            xb[nq][:, c, ts * 128:(ts + 1) * 128],
                            wv_all[:, c, :],
                            start=(c == 0), stop=(c == CC - 1),
                        )
                    nc.scalar.copy(out=vext[t][:, 0:128], in_=pv[:, 0:128])
                for h in range(GH):
                    pq = psum.tile([128, 512], f32, name="psq", tag="psP", bufs=2)
                    for c in range(CC):
                        nc.tensor.matmul(
                            pq, wq_all[:, c, h * 128:(h + 1) * 128], xb[nq][:, c, :],
                            start=(c == 0), stop=(c == CC - 1),
                        )
                    nc.scalar.copy(out=qT_blk[h][nq], in_=pq)

            for qc in range(NQ):
                emit_proj(qc)
                # ---- attention for query block qc (causal: kt tiles 0..4qc+3) ----
                nkt = 4 * qc + 4
                for h in range(GH):
                    pts = []
                    for kt in range(nkt):
                        # diagonal tiles: columns j < oi*128 are fully masked;
                        # compute only the live suffix [oi*128, 512)
                        oi = max(kt - 4 * qc, 0)
                        off = oi * 128
                        nw = 512 - off
                        pss = psum.tile([128, 512], f32, name="pss", tag="psS", bufs=2)
                        nc.tensor.matmul(
                            pss[:, 0:nw],
                            kT_blk[kt // 4][:, (kt % 4) * 128:(kt % 4 + 1) * 128],
                            qT_blk[h][qc][:, off:512],
                            start=True, stop=True,
                        )
                        pt = ptp.tile([128, 512], bf16, name="pt", tag="pt")
                        nc.scalar.activation(
                            out=pt[:, off:512], in_=pss[:, 0:nw],
                            func=mybir.ActivationFunctionType.Exp, scale=SCALE,
                        )
                        if kt >= 4 * qc:  # triangular mask on the partial block
                            tri = mask_sb[:, oi * 512 + off:oi * 512 + off + 128]
                            nc.vector.tensor_mul(
                                pt[:, off:off + 128], pt[:, off:off + 128], tri
                            )
                        pts.append(pt)
                    for sub in range(4):
                        qi = qc * 4 + sub
                        cps = psum.tile([128, 512], f32, name="cps", tag="psC", bufs=2)
                        for kt in range(qi + 1):
                            nc.tensor.matmul(
                                cps[:, 0:129],
                                pts[kt][:, sub * 128:(sub + 1) * 128],
                                vext[kt][:, 0:129],
                                start=(kt == 0), stop=(kt == qi),
                            )
                        rec = smalls.tile([128, 1], f32, name="rec", tag="rec")
                        nc.vector.reciprocal(rec, cps[:, 128:129])
                        cn = smalls.tile([128, 128], bf16, name="cn", tag="cn")
                        nc.vector.tensor_scalar_mul(cn, cps[:, 0:128], rec)
                        tp = psum.tile([128, 512], bf16, name="tp", tag="psC", bufs=2)
                        nc.tensor.transpose(tp[:, 0:128], cn, identity)
                        nc.vector.tensor_copy(
                            out=ctxT_blk[h][qc][:, sub * 128:(sub + 1) * 128], in_=tp[:, 0:128]
                        )


            # ---- out-projection, emitted last: global PE filler ----
            for tt in range(TT):
                osb = outsb.tile([128, D_IN], bf16, name="osb", tag="osb")
                for nch in range(NQ):
                    po = psum.tile([128, 512], f32, name="pso", tag="psO", bufs=2)
                    for h in range(GH):
                        nc.tensor.matmul(
                            po,
                            ctxT_blk[h][tt // 4][:, (tt % 4) * 128:(tt % 4 + 1) * 128],
                            wo_all[:, h, nch * 512:(nch + 1) * 512],
                            start=(h == 0), stop=(h == GH - 1),
                        )
                    nc.vector.tensor_copy(
                        out=osb[:, nch * 512:(nch + 1) * 512], in_=po
                    )
                nc.sync.dma_start(
                    out=out_d[tt * 128:(tt + 1) * 128, 0:1024], in_=osb[:, 0:1024]
                )
                nc.sync.dma_start(
                    out=out_d[tt * 128:(tt + 1) * 128, 1024:2048], in_=osb[:, 1024:2048]
                )

    nc.compile()
    return nc


def _get_compiled():
    global _COMPILED
    if _COMPILED is None:
        _COMPILED = _build()
    return _COMPILED


def _causal_mask():
    i = np.arange(128)[:, None]
    j = np.arange(512)[None, :]
    return np.concatenate(
        [(oi * 128 + i <= j) for oi in range(4)], axis=1
    ).astype(ml_dtypes.bfloat16)


def _pack_pmajor(w):
    # [CC*128, HD] -> [128, CC*HD]: out[p, c*HD+d] = w[c*128+p, d]
    return np.ascontiguousarray(
        w.reshape(CC, 128, -1).transpose(1, 0, 2).reshape(128, -1)
    )


def make_in_maps(x, Wq, Wk, Wv, Wo):
    bf16 = ml_dtypes.bfloat16
    x = np.asarray(x, np.float32)
    Wq = np.asarray(Wq, np.float32)
    Wk = np.asarray(Wk, np.float32)
    Wv = np.asarray(Wv, np.float32)
    Wo = np.asarray(Wo, np.float32)
    mask = _causal_mask()
    in_maps = []
    for core in range(8):
        bi, g = divmod(core, N_KV)
        in_maps.append({
            "xT": np.ascontiguousarray(x[bi].T).astype(bf16),
            "wq": np.ascontiguousarray(Wq[:, g * GD:(g + 1) * GD]).astype(bf16),
            "wk": _pack_pmajor(Wk[:, g * HD:(g + 1) * HD]).astype(bf16),
            "wv": _pack_pmajor(Wv[:, g * HD:(g + 1) * HD]).astype(bf16),
            "wo": np.ascontiguousarray(Wo[g * GD:(g + 1) * GD, :]).astype(bf16),
            "mask": mask,
        })
    return in_maps


def kernel(x, Wq, Wk, Wv, Wo, bo):
    from concourse.bass_utils import run_bass_kernel_spmd

    nc = _get_compiled()
    in_maps = make_in_maps(x, Wq, Wk, Wv, Wo)
    res = run_bass_kernel_spmd(nc, in_maps, core_ids=list(range(8)))
    out = np.zeros((B, T, D_IN), np.float32)
    for core in range(8):
        out[core // N_KV] += res.results[core]["out"]
    out += np.asarray(bo, np.float32)
    return out



# revision 2
# speedup vs baseline: 1.0699x; 1.0699x over previous
"""GQA attention block (16 query heads / 4 KV groups, head_dim 128) on 8 TRN2 NeuronCores.

Sharding: data-parallel over batch (b=2) x tensor-parallel over the 4 KV groups.
Core c handles batch c//4, KV group c%4 (4 query heads). Each core computes its
group's Q/K/V projections, causal softmax attention, and a partial out-projection
(row-shard of Wo); the host sums the 4 partials per batch and adds the bias.

Precision: the three projection GEMMs and the out-projection run as fp8-e4m3
hi/lo split matmuls in DoubleRow perf mode (A@B ~ Ah@Bh + Al@Bh + Ah@Bl, each
product pair packed into one DoubleRow instruction contracting 2x128 rows at
0.5 cycles/row -> 0.75x the bf16 PE cost at ~bf16 accuracy). Operands coming
from DRAM (x, Wq, Wk, Wv, Wo) are pre-scaled and split on the host; the ctx
hi/lo split happens on-chip after the per-block transposes. Scores and the
probs@V matmuls stay bf16 (no accumulation pairing available there, and fp8
probs would blow the error budget). All scale factors fold into activation
copies that exist anyway; the softmax denominator scale folds into the ones
column appended to V.

Schedule: inputs arrive packed, x column-block-first. Work streams per 512-row
query block - projections for block nq, then attention for query block nq
(which by causality only needs K/V blocks <= nq), then that block's
out-projection and output DMA. Projection matmuls of block nq+1 fill the PE
bubbles left by the exp->ctx latency chain of block nq.
"""

import math

import ml_dtypes
import numpy as np

B = 2
T = 2048
D_IN = 2048
N_KV = 4          # KV groups (one per core within a batch)
GH = 4            # query heads per KV group
HD = 128          # head dim
GD = GH * HD      # 512: per-group q/ctx width
TT = T // 128     # 16 row tiles
CC = D_IN // 128  # 16 contraction chunks
NQ = T // 512     # 4 query chunks of 512
SCALE = 1.0 / math.sqrt(HD)

# fp8 pre-scales (host) and their on-chip descales
SX = 8.0            # x ~ N(0,1) -> std 8
SW = 400.0          # W ~ N(0,0.02) -> std 8
SC = 16.0           # ctx (~std 0.2-0.9) -> comfortably in e4m3 range
SWO = 400.0
QK_DESCALE = 1.0 / (SX * SW)
OUT_DESCALE = 1.0 / (SC * SWO)

_COMPILED = None


def _build():
    import concourse.bacc as bacc
    import concourse.tile as tile
    from concourse import mybir
    from concourse.masks import make_identity

    bf16 = mybir.dt.bfloat16
    f8 = mybir.dt.float8e4
    f32 = mybir.dt.float32
    DR = mybir.MatmulPerfMode.DoubleRow

    nc = bacc.Bacc("TRN2", target_bir_lowering=False, debug=False)

    # host-prescaled fp8 hi/lo pairs; wk/wv packed partition-major: [128, c*HD]
    xh_d = nc.dram_tensor("xh", [D_IN, T], f8, kind="ExternalInput")
    xl_d = nc.dram_tensor("xl", [D_IN, T], f8, kind="ExternalInput")
    wqh_d = nc.dram_tensor("wqh", [D_IN, GD], f8, kind="ExternalInput")
    wql_d = nc.dram_tensor("wql", [D_IN, GD], f8, kind="ExternalInput")
    wkh_d = nc.dram_tensor("wkh", [128, CC * HD], f8, kind="ExternalInput")
    wkl_d = nc.dram_tensor("wkl", [128, CC * HD], f8, kind="ExternalInput")
    wvh_d = nc.dram_tensor("wvh", [128, CC * HD], f8, kind="ExternalInput")
    wvl_d = nc.dram_tensor("wvl", [128, CC * HD], f8, kind="ExternalInput")
    woh_d = nc.dram_tensor("woh", [GD, D_IN], f8, kind="ExternalInput")
    wol_d = nc.dram_tensor("wol", [GD, D_IN], f8, kind="ExternalInput")
    mask_d = nc.dram_tensor("mask", [128, 4 * 512], bf16, kind="ExternalInput")
    out_d = nc.dram_tensor("out", [T, D_IN], bf16, kind="ExternalOutput")

    with tile.TileContext(nc) as tc:
        with (
            tc.tile_pool(name="persist", bufs=1) as persist,
            tc.tile_pool(name="ptp", bufs=32) as ptp,
            tc.tile_pool(name="smalls", bufs=8) as smalls,
            tc.tile_pool(name="outsb", bufs=3) as outsb,
            tc.tile_pool(name="psum", bufs=2, space="PSUM") as psum,
        ):
            # ---- packed input DMAs, x column-block-first ----
            wkh = persist.tile([128, CC, HD], f8, name="wkh", tag="wkh")
            nc.sync.dma_start(out=wkh, in_=wkh_d.ap().rearrange("p (c n) -> p c n", c=CC))
            wkl = persist.tile([128, CC, HD], f8, name="wkl", tag="wkl")
            nc.sync.dma_start(out=wkl, in_=wkl_d.ap().rearrange("p (c n) -> p c n", c=CC))
            xbh = [
                persist.tile([128, CC, 512], f8, name=f"xbh{nq}", tag=f"xbh{nq}")
                for nq in range(NQ)
            ]
            xbl = [
                persist.tile([128, CC, 512], f8, name=f"xbl{nq}", tag=f"xbl{nq}")
                for nq in range(NQ)
            ]
            # split the first block's loads so kT/v matmuls can start after
            # the first half arrives (subtile deps unblock c-chunks 0..7)
            nc.sync.dma_start(
                out=xbh[0][:, 0:8, :],
                in_=xh_d[0:1024, 0:512].rearrange("(c p) n -> p c n", c=8),
            )
            nc.sync.dma_start(
                out=xbh[0][:, 8:16, :],
                in_=xh_d[1024:2048, 0:512].rearrange("(c p) n -> p c n", c=8),
            )
            nc.sync.dma_start(
                out=xbl[0][:, 0:8, :],
                in_=xl_d[0:1024, 0:512].rearrange("(c p) n -> p c n", c=8),
            )
            nc.sync.dma_start(
                out=xbl[0][:, 8:16, :],
                in_=xl_d[1024:2048, 0:512].rearrange("(c p) n -> p c n", c=8),
            )
            wvh = persist.tile([128, CC, HD], f8, name="wvh", tag="wvh")
            nc.sync.dma_start(out=wvh, in_=wvh_d.ap().rearrange("p (c n) -> p c n", c=CC))
            wvl = persist.tile([128, CC, HD], f8, name="wvl", tag="wvl")
            nc.sync.dma_start(out=wvl, in_=wvl_d.ap().rearrange("p (c n) -> p c n", c=CC))
            wqh = persist.tile([128, CC, GD], f8, name="wqh", tag="wqh")
            wql = persist.tile([128, CC, GD], f8, name="wql", tag="wql")
            # halves: heads 0-1 can project as soon as the first half lands
            nc.sync.dma_start(
                out=wqh[:, :, 0:256],
                in_=wqh_d[:, 0:256].rearrange("(c p) n -> p c n", c=CC),
            )
            nc.sync.dma_start(
                out=wqh[:, :, 256:512],
                in_=wqh_d[:, 256:512].rearrange("(c p) n -> p c n", c=CC),
            )
            nc.sync.dma_start(
                out=wql[:, :, 0:256],
                in_=wql_d[:, 0:256].rearrange("(c p) n -> p c n", c=CC),
            )
            nc.sync.dma_start(
                out=wql[:, :, 256:512],
                in_=wql_d[:, 256:512].rearrange("(c p) n -> p c n", c=CC),
            )
            mask_sb = persist.tile([128, 4 * 512], bf16, name="mask_sb", tag="mask_sb")
            nc.sync.dma_start(out=mask_sb, in_=mask_d[:, :])
            for t8 in (xh_d, xl_d):
                pass
            nc.sync.dma_start(
                out=xbh[1], in_=xh_d[:, 512:1024].rearrange("(c p) n -> p c n", c=CC)
            )
            nc.sync.dma_start(
                out=xbl[1], in_=xl_d[:, 512:1024].rearrange("(c p) n -> p c n", c=CC)
            )
            woh = persist.tile([128, GH, D_IN], f8, name="woh", tag="woh")
            nc.sync.dma_start(out=woh, in_=woh_d.ap().rearrange("(h p) n -> p h n", h=GH))
            wol = persist.tile([128, GH, D_IN], f8, name="wol", tag="wol")
            nc.sync.dma_start(out=wol, in_=wol_d.ap().rearrange("(h p) n -> p h n", h=GH))
            for nq in range(2, NQ):
                nc.sync.dma_start(
                    out=xbh[nq],
                    in_=xh_d[:, nq * 512:(nq + 1) * 512].rearrange(
                        "(c p) n -> p c n", c=CC
                    ),
                )
                nc.sync.dma_start(
                    out=xbl[nq],
                    in_=xl_d[:, nq * 512:(nq + 1) * 512].rearrange(
                        "(c p) n -> p c n", c=CC
                    ),
                )
            identity = persist.tile([128, 128], bf16, name="identity", tag="identity")
            make_identity(nc, identity)

            kT_blk = [
                persist.tile([128, 512], bf16, name=f"kT{nq}", tag=f"kT{nq}")
                for nq in range(NQ)
            ]
            qT_blk = [
                [
                    persist.tile([128, 512], bf16, name=f"qT{h}_{nq}", tag=f"qT{h}_{nq}")
                    for nq in range(NQ)
                ]
                for h in range(GH)
            ]
            vext = [
                persist.tile([128, 132], bf16, name=f"vx{t}", tag=f"vx{t}")
                for t in range(TT)
            ]
            for t in range(TT):
                # ones column scaled by 1/SC: denominator comes out as sum(p)/SC,
                # so the reciprocal-normalize leaves ctx scaled by SC for the
                # fp8 hi/lo split ahead of the out-projection.
                nc.vector.memset(vext[t][:, 128:129], 1.0 / SC)
            # per-qc transposed ctx, fp8 hi/lo, head-major for DoubleRow pairing
            ctxTh = [
                persist.tile([128, GH, 512], f8, name=f"cTh{nq}", tag=f"cTh{nq}")
                for nq in range(NQ)
            ]
            ctxTl = [
                persist.tile([128, GH, 512], f8, name=f"cTl{nq}", tag=f"cTl{nq}")
                for nq in range(NQ)
            ]

            def dr_chain(ps, pairs):
                """Emit a DoubleRow accumulation chain into psum tile ps.
                pairs: list of (lhsT_ap, rhs_ap) already shaped [128, 2, *]."""
                n = len(pairs)
                for i, (lt, rt) in enumerate(pairs):
                    nc.tensor.matmul(
                        ps, lt, rt,
                        start=(i == 0), stop=(i == n - 1), perf_mode=DR,
                    )

            def proj_pairs(wh, wl, xh, xl, wslc, xslc):
                # 3-term hi/lo: Wh.T@Xh + Wl.T@Xh + Wh.T@Xl, chunk-paired
                out = []
                for wt, xt in ((wh, xh), (wl, xh), (wh, xl)):
                    for c in range(0, CC, 2):
                        out.append((wt[:, c:c + 2, wslc], xt[:, c:c + 2, xslc]))
                return out

            full = slice(0, None)

            def emit_proj(nq):
                ps = psum.tile([128, 512], f32, name="pskt", tag="psP", bufs=2)
                dr_chain(ps, proj_pairs(wkh, wkl, xbh[nq], xbl[nq], full, full))
                nc.scalar.mul(kT_blk[nq], ps, QK_DESCALE)
                for ts in range(4):
                    t = nq * 4 + ts
                    tslc = slice(ts * 128, (ts + 1) * 128)
                    pv = psum.tile([128, 512], f32, name="psv", tag="psP", bufs=2)
                    # v: lhsT = x (tokens stationary), rhs = wv
                    pairs = []
                    for xt, wt in ((xbh[nq], wvh), (xbl[nq], wvh), (xbh[nq], wvl)):
                        for c in range(0, CC, 2):
                            pairs.append((xt[:, c:c + 2, tslc], wt[:, c:c + 2, :]))
                    dr_chain(pv[:, 0:128], pairs)
                    nc.scalar.mul(vext[t][:, 0:128], pv[:, 0:128], QK_DESCALE)
                for h in range(GH):
                    hslc = slice(h * 128, (h + 1) * 128)
                    pq = psum.tile([128, 512], f32, name="psq", tag="psP", bufs=2)
                    dr_chain(pq, proj_pairs(wqh, wql, xbh[nq], xbl[nq], hslc, full))
                    nc.scalar.mul(qT_blk[h][nq], pq, QK_DESCALE)

            for qc in range(NQ):
                emit_proj(qc)
                # ---- attention for query block qc (causal: kt tiles 0..4qc+3) ----
                nkt = 4 * qc + 4
                for h in range(GH):
                    pts = []
                    for kt in range(nkt):
                        # diagonal tiles: columns j < oi*128 are fully masked;
                        # compute only the live suffix [oi*128, 512)
                        oi = max(kt - 4 * qc, 0)
                        off = oi * 128
                        nw = 512 - off
                        pss = psum.tile([128, 512], f32, name="pss", tag="psS", bufs=2)
                        nc.tensor.matmul(
                            pss[:, 0:nw],
                            kT_blk[kt // 4][:, (kt % 4) * 128:(kt % 4 + 1) * 128],
                            qT_blk[h][qc][:, off:512],
                            start=True, stop=True,
                        )
                        pt = ptp.tile([128, 512], bf16, name="pt", tag="pt")
                        nc.scalar.activation(
                            out=pt[:, off:512], in_=pss[:, 0:nw],
                            func=mybir.ActivationFunctionType.Exp, scale=SCALE,
                        )
                        if kt >= 4 * qc:  # triangular mask on the partial block
                            tri = mask_sb[:, oi * 512 + off:oi * 512 + off + 128]
                            nc.vector.tensor_mul(
                                pt[:, off:off + 128], pt[:, off:off + 128], tri
                            )
                        pts.append(pt)
                    for sub in range(4):
                        qi = qc * 4 + sub
                        cps = psum.tile([128, 512], f32, name="cps", tag="psC", bufs=2)
                        for kt in range(qi + 1):
                            nc.tensor.matmul(
                                cps[:, 0:129],
                                pts[kt][:, sub * 128:(sub + 1) * 128],
                                vext[kt][:, 0:129],
                                start=(kt == 0), stop=(kt == qi),
                            )
                        rec = smalls.tile([128, 1], f32, name="rec", tag="rec")
                        nc.vector.reciprocal(rec, cps[:, 128:129])
                        cn = smalls.tile([128, 128], bf16, name="cn", tag="cn")
                        # cn = ctx * SC (SC folded in via the scaled ones column)
                        nc.vector.tensor_scalar_mul(cn, cps[:, 0:128], rec)
                        tp = psum.tile([128, 512], bf16, name="tp", tag="psC", bufs=2)
                        nc.tensor.transpose(tp[:, 0:128], cn, identity)
                        sslc = slice(sub * 128, (sub + 1) * 128)
                        nc.scalar.copy(ctxTh[qc][:, h, sslc], tp[:, 0:128])
                        nc.vector.tensor_sub(
                            ctxTl[qc][:, h, sslc], tp[:, 0:128], ctxTh[qc][:, h, sslc]
                        )

            # ---- out-projection, emitted last: global PE filler ----
            for tt in range(TT):
                osb = outsb.tile([128, D_IN], bf16, name="osb", tag="osb")
                tslc = slice((tt % 4) * 128, (tt % 4 + 1) * 128)
                for nch in range(NQ):
                    po = psum.tile([128, 512], f32, name="pso", tag="psO", bufs=2)
                    oslc = slice(nch * 512, (nch + 1) * 512)
                    pairs = []
                    for ct, wt in (
                        (ctxTh[tt // 4], woh),
                        (ctxTl[tt // 4], woh),
                        (ctxTh[tt // 4], wol),
                    ):
                        for hp in range(0, GH, 2):
                            pairs.append((ct[:, hp:hp + 2, tslc], wt[:, hp:hp + 2, oslc]))
                    dr_chain(po, pairs)
                    nc.vector.tensor_scalar_mul(osb[:, oslc], po, OUT_DESCALE)
                nc.sync.dma_start(
                    out=out_d[tt * 128:(tt + 1) * 128, 0:1024], in_=osb[:, 0:1024]
                )
                nc.sync.dma_start(
                    out=out_d[tt * 128:(tt + 1) * 128, 1024:2048], in_=osb[:, 1024:2048]
                )

    nc.compile()
    return nc


def _get_compiled():
    global _COMPILED
    if _COMPILED is None:
        _COMPILED = _build()
    return _COMPILED


def _causal_mask():
    i = np.arange(128)[:, None]
    j = np.arange(512)[None, :]
    return np.concatenate(
        [(oi * 128 + i <= j) for oi in range(4)], axis=1
    ).astype(ml_dtypes.bfloat16)


def _pack_pmajor(w):
    # [CC*128, HD] -> [128, CC*HD]: out[p, c*HD+d] = w[c*128+p, d]
    return np.ascontiguousarray(
        w.reshape(CC, 128, -1).transpose(1, 0, 2).reshape(128, -1)
    )


def _hilo(a, scale):
    f8 = ml_dtypes.float8_e4m3
    s = np.asarray(a, np.float32) * scale
    hi = s.astype(f8)
    lo = (s - hi.astype(np.float32)).astype(f8)
    return np.ascontiguousarray(hi), np.ascontiguousarray(lo)


def make_in_maps(x, Wq, Wk, Wv, Wo):
    x = np.asarray(x, np.float32)
    Wq = np.asarray(Wq, np.float32)
    Wk = np.asarray(Wk, np.float32)
    Wv = np.asarray(Wv, np.float32)
    Wo = np.asarray(Wo, np.float32)
    mask = _causal_mask()
    in_maps = []
    for core in range(8):
        bi, g = divmod(core, N_KV)
        xh, xl = _hilo(x[bi].T, SX)
        wqh, wql = _hilo(Wq[:, g * GD:(g + 1) * GD], SW)
        wkh, wkl = _hilo(_pack_pmajor(Wk[:, g * HD:(g + 1) * HD]), SW)
        wvh, wvl = _hilo(_pack_pmajor(Wv[:, g * HD:(g + 1) * HD]), SW)
        woh, wol = _hilo(Wo[g * GD:(g + 1) * GD, :], SWO)
        in_maps.append({
            "xh": xh, "xl": xl,
            "wqh": wqh, "wql": wql,
            "wkh": wkh, "wkl": wkl,
            "wvh": wvh, "wvl": wvl,
            "woh": woh, "wol": wol,
            "mask": mask,
        })
    return in_maps


def kernel(x, Wq, Wk, Wv, Wo, bo):
    from concourse.bass_utils import run_bass_kernel_spmd

    nc = _get_compiled()
    in_maps = make_in_maps(x, Wq, Wk, Wv, Wo)
    res = run_bass_kernel_spmd(nc, in_maps, core_ids=list(range(8)))
    out = np.zeros((B, T, D_IN), np.float32)
    for core in range(8):
        out[core // N_KV] += res.results[core]["out"]
    out += np.asarray(bo, np.float32)
    return out
